# revision 51
# baseline (speedup 1.0000x reference)
"""Causal multi-head attention on 8 TRN2 NeuronCores (v2).

Problem: B=2, L=2048, H=16, E=64 (f32 in/out). B*H = 32 (batch, head)
slices are data-parallel: 4 slices per core, no cross-core comm.

Per-core algorithm (per slice, matmul operands bf16, PSUM f32):
  - l-window-outer schedule: window lp covers l in [512lp, 512lp+512).
  - S^T[m, l] = K^T Q per 128x128 causal block (mi <= li), runs grouped
    into <=3-PSUM-bank groups within a window.
  - P^T = exp(S^T * scale): split between ScalarE (exact activation) and
    VectorE (Schraudolph bit trick: int16(S*c1 + c2) bitcast to bf16,
    one tensor_scalar op) to break the ScalarE exp throughput wall.
  - diagonal blocks masked in-place with gpsimd affine_select.
  - O accumulation V-stationary: out[e|den, l-window] += V(mi)^T P(mi)
    into one PSUM bank per window; V carries a ones column so row 64
    accumulates the softmax denominator.
  - evacuate [65, 512] windows to SBUF as bf16 (VectorE/ScalarE
    alternating), one DMA per slice to DRAM.
  - normalization (divide by denominator row) + transpose on host.

Schedule: groups flow through a depth-3 software pipeline (psS is
triple-buffered) so the PE always has S matmuls queued while exps run;
input DMAs are staged and prefetched one slice ahead; dummy warmup
matmuls hold the PE busy through the HAM activity window so everything
runs at 2.4 GHz.
"""

import numpy as np
import ml_dtypes
from contextlib import ExitStack

import concourse.bass as bass
import concourse.mybir as mybir
import concourse.tile as tile
from concourse import bacc
from concourse.bass_utils import run_bass_kernel_spmd

B, L, H, E = 2, 2048, 16, 64
N_CORES = 8
NS = (B * H) // N_CORES  # slices per core = 4
NT = L // 128  # 16 tiles of 128 along both l and m
F32 = mybir.dt.float32
BF16 = mybir.dt.bfloat16
I16 = mybir.dt.int16
BF16NP = ml_dtypes.bfloat16

PAIR_S = True  # K=64 row-group-paired S matmuls (2 concurrent on PE halves)
DVE_FRAC = 0.42  # target fraction of exp columns on VectorE bit trick
GROUP_BANKS = 2  # PSUM banks per S group (psS bufs * GROUP_BANKS + 2 <= 8)
PIPE_DEPTH = 3  # S groups emitted ahead of their exp/O flush
ADJ = 0.0573  # Schraudolph centering (octave units)
LOG2E = 1.4426950408889634

# S in PSUM is 2*S when K is duplicated to 128 rows (full-K matmul), 1*S
# with K=64 pairs. exp argument must be S/8.
S_FACTOR = 1.0 if PAIR_S else 2.0
SCALE_ACT = 0.125 / S_FACTOR
C1 = 128.0 * LOG2E * SCALE_ACT
C2 = 128.0 * (127.0 - ADJ)


def _plan():
    """Static per-slice schedule, window (lp) outer.

    Window lp consumes l-blocks li in [4lp, 4lp+4). Run (mi, lp) covers
    S blocks (mi, li) for li in [max(mi, 4lp), 4lp+4) — n = 4lp+4-l0
    blocks, always extending to the window end. Runs are first-fit
    bank-packed (each PSUM bank holds <=4 128-col blocks; only the last
    bank of a window is partial) so activation inputs are contiguous.
    Banks are chunked into groups of <=3 (one psS tile); each group is
    exp'd by ScalarE or VectorE per a running-balance assignment.
    """
    windows = []
    pt_base = 0
    dve_cols = 0
    tot_cols = 0
    for lp in range(4):
        runs = []
        bank_fill = []
        for mi in range(4 * lp + 4):
            l0 = max(mi, 4 * lp)
            n = 4 * lp + 4 - l0
            bk = None
            for bi, f in enumerate(bank_fill):
                if f + n <= 4:
                    bk = bi
                    break
            if bk is None:
                bk = len(bank_fill)
                bank_fill.append(0)
            runs.append(
                {"mi": mi, "l0": l0, "n": n, "bank": bk,
                 "off": 128 * bank_fill[bk], "diag": l0 == mi}
            )
            bank_fill[bk] += n
        nbanks = len(bank_fill)
        groups = []
        for b0 in range(0, nbanks, GROUP_BANKS):
            b1 = min(b0 + GROUP_BANKS, nbanks)
            gruns = [r for r in runs if b0 <= r["bank"] < b1]
            for r in gruns:
                r["ps_col"] = 512 * (r["bank"] - b0) + r["off"]
                r["pt_col"] = pt_base + r["ps_col"]
            cols = max(r["ps_col"] + 128 * r["n"] for r in gruns)
            # greedy running balance toward DVE_FRAC
            if (dve_cols + cols) <= DVE_FRAC * (tot_cols + cols):
                eng = "dve"
                dve_cols += cols
            else:
                eng = "act"
            tot_cols += cols
            # keep emission (and O accumulation) in mi order
            gruns = sorted(gruns, key=lambda r: r["mi"])
            groups.append({"runs": gruns, "cols": cols, "eng": eng,
                           "pt_base": pt_base, "lp": lp})
            pt_base += cols
        # mark the final run in window emission order (group-major, then
        # mi): it carries stop=True and triggers the window evacuation
        for g in groups:
            for r in g["runs"]:
                r["o_last"] = False
        groups[-1]["runs"][-1]["o_last"] = True
        windows.append(groups)
    # force the slice's last groups onto ScalarE so the next slice's
    # first S matmuls don't stall on a backed-up VectorE queue
    for g in windows[-1][-3:]:
        g["eng"] = "act"
    return windows, pt_base


WINDOWS, PT_COLS = _plan()


def _emit_warmup(tc, pools):
    """Dummy matmuls on scratch data during the initial input-DMA wait:
    keeps the PE busy through one full HAM activity window so real work
    runs at 2.4 GHz from the start instead of warming up ~12us in."""
    nc = tc.nc
    (io_q, io_k, io_v, pt_pool, o_pool, psS, psO) = pools
    scratch = io_v.tile([128, 512], BF16, name="warm", tag="warm")
    nc.gpsimd.memset(scratch[:, :], 0.0)
    ps = psS.tile([128, GROUP_BANKS * 512], F32, name="ps", tag="ps")
    for i in range(9):
        nc.tensor.matmul(
            ps[:, 512 * (i % 2) : 512 * (i % 2) + 512],
            lhsT=scratch[:, 0:128],
            rhs=scratch[:, :],
            start=True,
            stop=True,
            skip_group_check=True,
        )
    return scratch


def _emit_loads(tc, pools, qT, kT, v, s):
    """Input loads for slice s. Emitted one slice ahead of its compute so
    the v memset (gpsimd) and DMA descriptors (sync queue) are not stuck
    behind the previous slice's affine_selects / output DMAs."""
    nc = tc.nc
    (io_q, io_k, io_v, pt_pool, o_pool, psS, psO) = pools

    # Q^T/K^T duplicated into both partition halves (PAIR_S: the halves
    # feed two concurrent K=64 matmuls on disjoint PE row groups).
    # Slice 0 is staged by l/m range (window lp only touches columns
    # below 512(lp+1)) so compute starts ~5us earlier; later slices
    # prefetch during the previous slice's compute, where fewer DMA
    # instructions win (descriptor generation is ~0.7us per DMA).
    # The strided v load (many small packets, slow) goes after the first
    # Q/K chunk: matmuls need Q/K immediately, v only at the first exp.
    qT_sb = io_q.tile([128, L], BF16)
    kT_sb = io_k.tile([128, L], BF16)
    v_sb = io_v.tile([128, NT * 65], BF16)
    nc.gpsimd.memset(v_sb[:, :], 1.0)
    chunks = ((0, 512), (512, L)) if s == 0 else ((0, L),)
    for c0, c1 in chunks:
        for dst, src in ((kT_sb, kT), (qT_sb, qT)):
            nc.sync.dma_start(dst[0:E, c0:c1], src[s][:, c0:c1])
            nc.sync.dma_start(dst[E:128, c0:c1], src[s][:, c0:c1])
    # v goes last: its ~1.4us descriptor generation would otherwise delay
    # the stage-B Q/K chunks, and the first O matmul needs v only after
    # the first exp completes
    v_src = v[s].rearrange("(t p) e -> p t e", p=128)
    v_dst = v_sb.rearrange("p (t x) -> p t x", t=NT, x=65)[:, :, 0:E]
    nc.sync.dma_start(v_dst, v_src)
    return qT_sb, kT_sb, v_sb


def _emit_slice(tc, pools, loads, outT, s, scratch=None):
    nc = tc.nc
    (io_q, io_k, io_v, pt_pool, o_pool, psS, psO) = pools
    qT_sb, kT_sb, v_sb = loads

    pT = pt_pool.tile([128, PT_COLS], BF16)

    state = {"po": None, "o_t": None}

    def emit_s(g):
        ps = psS.tile([128, GROUP_BANKS * 512], F32, name="ps", tag="ps")
        g["ps"] = ps
        for r in g["runs"]:
            mi = r["mi"]
            if PAIR_S:
                h0 = 0 if mi % 2 == 0 else 64
                lhsT = kT_sb[h0 : h0 + 64, 128 * mi : 128 * mi + 128]
                rhs = qT_sb[h0 : h0 + 64, 128 * r["l0"] : 128 * (r["l0"] + r["n"])]
            else:
                lhsT = kT_sb[:, 128 * mi : 128 * mi + 128]
                rhs = qT_sb[:, 128 * r["l0"] : 128 * (r["l0"] + r["n"])]
            nc.tensor.matmul(
                ps[:, r["ps_col"] : r["ps_col"] + 128 * r["n"]],
                lhsT=lhsT,
                rhs=rhs,
                start=True,
                stop=True,
            )

    def emit_po(g):
        ps = g["ps"]
        dst = pT[:, g["pt_base"] : g["pt_base"] + g["cols"]]
        if g["eng"] == "act":
            nc.scalar.activation(
                dst, ps[:, : g["cols"]],
                mybir.ActivationFunctionType.Exp, scale=SCALE_ACT,
            )
        else:
            nc.vector.tensor_scalar(
                dst.bitcast(I16), ps[:, : g["cols"]],
                C1, C2, mybir.AluOpType.mult, mybir.AluOpType.add,
            )
        for r in g["runs"]:
            if r["diag"]:
                seg = pT[:, r["pt_col"] : r["pt_col"] + 128]
                nc.gpsimd.affine_select(
                    out=seg,
                    in_=seg,
                    pattern=[[1, 128]],
                    compare_op=mybir.AluOpType.is_ge,
                    fill=0.0,
                    base=0,
                    channel_multiplier=-1,
                )
        lp = g["lp"]
        for r in g["runs"]:
            mi = r["mi"]
            if mi == 0:
                state["po"] = psO.tile([128, 512], F32, name="po", tag="po")
            po = state["po"]
            odst = 128 * (r["l0"] - 4 * lp)
            nc.tensor.matmul(
                po[0:65, odst:512],
                lhsT=v_sb[:, 65 * mi : 65 * mi + 65],
                rhs=pT[:, r["pt_col"] : r["pt_col"] + 128 * r["n"]],
                start=(mi == 0),
                stop=r["o_last"],
                skip_group_check=True,
            )
            if r["o_last"]:
                if state["o_t"] is None:
                    state["o_t"] = o_pool.tile([128, L], BF16, name="ot", tag="ot")
                o_t = state["o_t"]
                # alternate the evacuation engine so neither queue backs up
                # (VectorE for the kernel's final window: its queue is
                # empty there, ScalarE still drains the forced-act exps)
                if lp % 2 == 0 or (s == NS - 1 and lp == 3):
                    nc.vector.tensor_copy(
                        o_t[0:65, 512 * lp : 512 * lp + 512], po[0:65, :]
                    )
                else:
                    nc.scalar.copy(
                        o_t[0:65, 512 * lp : 512 * lp + 512], po[0:65, :]
                    )
                if s == NS - 1:
                    # last slice: ship each window as it completes so the
                    # final DMA tail is one small window, not the slice
                    nc.sync.dma_start(
                        outT[s][:, 512 * lp : 512 * lp + 512],
                        o_t[0:65, 512 * lp : 512 * lp + 512],
                    )
                elif lp == 3:
                    nc.sync.dma_start(outT[s], o_t[0:65, :])

    # software pipeline: keep PIPE_DEPTH groups of S matmuls queued ahead
    # of the current group's exp/O flush so PE never waits on ScalarE
    pending = []
    nemit = 0
    for groups in WINDOWS:
        for g in groups:
            emit_s(g)
            nemit += 1
            if s == 0 and scratch is not None and nemit == PIPE_DEPTH:
                # fill slice 0's first-exp latency with dummy matmuls so
                # the HAM activity window never sees the PE idle
                po_d = psO.tile([128, 512], F32, name="po", tag="po")
                for _ in range(4):
                    nc.tensor.matmul(
                        po_d[:, :],
                        lhsT=scratch[:, 0:128],
                        rhs=scratch[:, :],
                        start=True,
                        stop=True,
                    )
            pending.append(g)
            if len(pending) > PIPE_DEPTH:
                emit_po(pending.pop(0))
    for g in pending:
        emit_po(g)


def _build():
    nc = bacc.Bacc(
        "TRN2",
        target_bir_lowering=False,
        debug=False,
        enable_asserts=True,
        num_devices=N_CORES,
    )
    qT = nc.dram_tensor("qT", [NS, E, L], BF16, kind="ExternalInput").ap()
    kT = nc.dram_tensor("kT", [NS, E, L], BF16, kind="ExternalInput").ap()
    v = nc.dram_tensor("v", [NS, L, E], BF16, kind="ExternalInput").ap()
    outT = nc.dram_tensor("outT", [NS, E + 1, L], BF16, kind="ExternalOutput").ap()

    with tile.TileContext(nc) as tc:
        with ExitStack() as ctx:

            def pool(name, bufs, space="SBUF"):
                return ctx.enter_context(
                    tc.tile_pool(name=name, bufs=bufs, space=space)
                )

            pools = (
                pool("io_q", 2),
                pool("io_k", 2),
                pool("io_v", 2),
                pool("pt", 2),
                pool("o", 2),
                pool("psS", 3, "PSUM"),
                pool("psO", 2, "PSUM"),
            )
            scratch = _emit_warmup(tc, pools)
            loads = _emit_loads(tc, pools, qT, kT, v, 0)
            for s in range(NS):
                nxt = (
                    _emit_loads(tc, pools, qT, kT, v, s + 1)
                    if s + 1 < NS else None
                )
                _emit_slice(tc, pools, loads, outT, s,
                            scratch if s == 0 else None)
                loads = nxt

    nc.compile()
    return nc


_NC_CACHE = {}


def _get_nc():
    if "nc" not in _NC_CACHE:
        _NC_CACHE["nc"] = _build()
    return _NC_CACHE["nc"]


def kernel(queries, keys, values, trace=False, tmpdir=None):
    nc = _get_nc()

    # shard: slice g = b*H + h; per-core slices [4c, 4c+4)
    qTf = np.ascontiguousarray(
        queries.transpose(0, 2, 3, 1).reshape(B * H, E, L)
    ).astype(BF16NP)
    kTf = np.ascontiguousarray(
        keys.transpose(0, 2, 3, 1).reshape(B * H, E, L)
    ).astype(BF16NP)
    vf = np.ascontiguousarray(
        values.transpose(0, 2, 1, 3).reshape(B * H, L, E)
    ).astype(BF16NP)

    in_maps = [
        {
            "qT": qTf[NS * c : NS * (c + 1)],
            "kT": kTf[NS * c : NS * (c + 1)],
            "v": vf[NS * c : NS * (c + 1)],
        }
        for c in range(N_CORES)
    ]

    res = run_bass_kernel_spmd(
        nc, in_maps, core_ids=list(range(N_CORES)), trace=trace, tmpdir=tmpdir
    )

    outT = np.concatenate([res.results[c]["outT"] for c in range(N_CORES)], axis=0)
    outT = outT.astype(np.float32)
    # outT: [B*H, 65, L]: rows 0..63 = O^T (unnormalized), row 64 = denom
    out = outT[:, :E, :] / outT[:, E : E + 1, :]
    # [B*H, E, L] -> [B, L, H, E]
    out = out.reshape(B, H, E, L).transpose(0, 3, 1, 2)
    out = np.ascontiguousarray(out, dtype=np.float32)
    if trace:
        kernel.last_exec_time_ns = res.exec_time_ns
    return out


# revision 52
# speedup vs baseline: 1.0080x; 1.0080x over previous
"""Causal multi-head attention on 8 TRN2 NeuronCores (v2).

Problem: B=2, L=2048, H=16, E=64 (f32 in/out). B*H = 32 (batch, head)
slices are data-parallel: 4 slices per core, no cross-core comm.

Per-core algorithm (per slice, matmul operands bf16, PSUM f32):
  - l-window-outer schedule: window lp covers l in [512lp, 512lp+512).
  - S^T[m, l] = K^T Q per 128x128 causal block (mi <= li), runs grouped
    into <=3-PSUM-bank groups within a window.
  - P^T = exp(S^T * scale): split between ScalarE (exact activation) and
    VectorE (Schraudolph bit trick: int16(S*c1 + c2) bitcast to bf16,
    one tensor_scalar op) to break the ScalarE exp throughput wall.
  - diagonal blocks masked in-place with gpsimd affine_select.
  - O accumulation V-stationary: out[e|den, l-window] += V(mi)^T P(mi)
    into one PSUM bank per window; V carries a ones column so row 64
    accumulates the softmax denominator.
  - evacuate [65, 512] windows to SBUF as bf16 (VectorE/ScalarE
    alternating), one DMA per slice to DRAM.
  - normalization (divide by denominator row) + transpose on host.

Schedule: groups flow through a depth-3 software pipeline (psS is
triple-buffered) so the PE always has S matmuls queued while exps run;
input DMAs are staged and prefetched one slice ahead; dummy warmup
matmuls hold the PE busy through the HAM activity window so everything
runs at 2.4 GHz.
"""

import numpy as np
import ml_dtypes
from contextlib import ExitStack

import concourse.bass as bass
import concourse.mybir as mybir
import concourse.tile as tile
from concourse import bacc
from concourse.bass_utils import run_bass_kernel_spmd

B, L, H, E = 2, 2048, 16, 64
N_CORES = 8
NS = (B * H) // N_CORES  # slices per core = 4
NT = L // 128  # 16 tiles of 128 along both l and m
F32 = mybir.dt.float32
BF16 = mybir.dt.bfloat16
I16 = mybir.dt.int16
BF16NP = ml_dtypes.bfloat16

PAIR_S = True  # K=64 row-group-paired S matmuls (2 concurrent on PE halves)
DVE_FRAC = 0.42  # target fraction of exp columns on VectorE bit trick
GROUP_BANKS = 2  # PSUM banks per S group (psS bufs * GROUP_BANKS + 2 <= 8)
PIPE_DEPTH = 3  # S groups emitted ahead of their exp/O flush
ADJ = 0.0573  # Schraudolph centering (octave units)
LOG2E = 1.4426950408889634

# S in PSUM is 2*S when K is duplicated to 128 rows (full-K matmul), 1*S
# with K=64 pairs. exp argument must be S/8.
S_FACTOR = 1.0 if PAIR_S else 2.0
SCALE_ACT = 0.125 / S_FACTOR
C1 = 128.0 * LOG2E * SCALE_ACT
C2 = 128.0 * (127.0 - ADJ)


def _plan():
    """Static per-slice schedule, window (lp) outer.

    Window lp consumes l-blocks li in [4lp, 4lp+4). Run (mi, lp) covers
    S blocks (mi, li) for li in [max(mi, 4lp), 4lp+4) — n = 4lp+4-l0
    blocks, always extending to the window end. Runs are first-fit
    bank-packed (each PSUM bank holds <=4 128-col blocks; only the last
    bank of a window is partial) so activation inputs are contiguous.
    Banks are chunked into groups of <=3 (one psS tile); each group is
    exp'd by ScalarE or VectorE per a running-balance assignment.
    """
    windows = []
    pt_base = 0
    dve_cols = 0
    tot_cols = 0
    for lp in range(4):
        runs = []
        bank_fill = []
        for mi in range(4 * lp + 4):
            l0 = max(mi, 4 * lp)
            n = 4 * lp + 4 - l0
            bk = None
            for bi, f in enumerate(bank_fill):
                if f + n <= 4:
                    bk = bi
                    break
            if bk is None:
                bk = len(bank_fill)
                bank_fill.append(0)
            runs.append(
                {"mi": mi, "l0": l0, "n": n, "bank": bk,
                 "off": 128 * bank_fill[bk], "diag": l0 == mi}
            )
            bank_fill[bk] += n
        nbanks = len(bank_fill)
        groups = []
        for b0 in range(0, nbanks, GROUP_BANKS):
            b1 = min(b0 + GROUP_BANKS, nbanks)
            gruns = [r for r in runs if b0 <= r["bank"] < b1]
            for r in gruns:
                r["ps_col"] = 512 * (r["bank"] - b0) + r["off"]
                r["pt_col"] = pt_base + r["ps_col"]
            cols = max(r["ps_col"] + 128 * r["n"] for r in gruns)
            # greedy running balance toward DVE_FRAC
            if (dve_cols + cols) <= DVE_FRAC * (tot_cols + cols):
                eng = "dve"
                dve_cols += cols
            else:
                eng = "act"
            tot_cols += cols
            # keep emission (and O accumulation) in mi order
            gruns = sorted(gruns, key=lambda r: r["mi"])
            groups.append({"runs": gruns, "cols": cols, "eng": eng,
                           "pt_base": pt_base, "lp": lp})
            pt_base += cols
        # mark the final run in window emission order (group-major, then
        # mi): it carries stop=True and triggers the window evacuation
        for g in groups:
            for r in g["runs"]:
                r["o_last"] = False
        groups[-1]["runs"][-1]["o_last"] = True
        windows.append(groups)
    # force the slice's last groups onto ScalarE so the next slice's
    # first S matmuls don't stall on a backed-up VectorE queue
    for g in windows[-1][-3:]:
        g["eng"] = "act"
    return windows, pt_base


WINDOWS, PT_COLS = _plan()


def _emit_warmup(tc, pools):
    """Dummy matmuls on scratch data during the initial input-DMA wait:
    keeps the PE busy through one full HAM activity window so real work
    runs at 2.4 GHz from the start instead of warming up ~12us in."""
    nc = tc.nc
    (io_q, io_k, io_v, pt_pool, o_pool, psS, psO) = pools
    scratch = io_v.tile([128, 512], BF16, name="warm", tag="warm")
    nc.gpsimd.memset(scratch[:, :], 0.0)
    ps = psS.tile([128, GROUP_BANKS * 512], F32, name="ps", tag="ps")
    for i in range(12):
        nc.tensor.matmul(
            ps[:, 512 * (i % 2) : 512 * (i % 2) + 512],
            lhsT=scratch[:, 0:128],
            rhs=scratch[:, :],
            start=True,
            stop=True,
            skip_group_check=True,
        )
    return scratch


def _emit_loads(tc, pools, qT, kT, v, s):
    """Input loads for slice s. Emitted one slice ahead of its compute so
    the v memset (gpsimd) and DMA descriptors (sync queue) are not stuck
    behind the previous slice's affine_selects / output DMAs."""
    nc = tc.nc
    (io_q, io_k, io_v, pt_pool, o_pool, psS, psO) = pools

    # Q^T/K^T duplicated into both partition halves (PAIR_S: the halves
    # feed two concurrent K=64 matmuls on disjoint PE row groups).
    # Slice 0 is staged by l/m range (window lp only touches columns
    # below 512(lp+1)) so compute starts ~5us earlier; later slices
    # prefetch during the previous slice's compute, where fewer DMA
    # instructions win (descriptor generation is ~0.7us per DMA).
    # The strided v load (many small packets, slow) goes after the first
    # Q/K chunk: matmuls need Q/K immediately, v only at the first exp.
    qT_sb = io_q.tile([128, L], BF16)
    kT_sb = io_k.tile([128, L], BF16)
    v_sb = io_v.tile([128, NT * 65], BF16)
    nc.gpsimd.memset(v_sb[:, :], 1.0)
    chunks = ((0, 512), (512, L)) if s == 0 else ((0, L),)
    for c0, c1 in chunks:
        for dst, src in ((kT_sb, kT), (qT_sb, qT)):
            nc.sync.dma_start(dst[0:E, c0:c1], src[s][:, c0:c1])
            nc.sync.dma_start(dst[E:128, c0:c1], src[s][:, c0:c1])
    # v goes last: its ~1.4us descriptor generation would otherwise delay
    # the stage-B Q/K chunks, and the first O matmul needs v only after
    # the first exp completes
    v_src = v[s].rearrange("(t p) e -> p t e", p=128)
    v_dst = v_sb.rearrange("p (t x) -> p t x", t=NT, x=65)[:, :, 0:E]
    nc.sync.dma_start(v_dst, v_src)
    return qT_sb, kT_sb, v_sb


def _emit_slice(tc, pools, loads, outT, s, scratch=None):
    nc = tc.nc
    (io_q, io_k, io_v, pt_pool, o_pool, psS, psO) = pools
    qT_sb, kT_sb, v_sb = loads

    pT = pt_pool.tile([128, PT_COLS], BF16)

    state = {"po": None, "o_t": None}

    def emit_s(g):
        ps = psS.tile([128, GROUP_BANKS * 512], F32, name="ps", tag="ps")
        g["ps"] = ps
        for r in g["runs"]:
            mi = r["mi"]
            if PAIR_S:
                h0 = 0 if mi % 2 == 0 else 64
                lhsT = kT_sb[h0 : h0 + 64, 128 * mi : 128 * mi + 128]
                rhs = qT_sb[h0 : h0 + 64, 128 * r["l0"] : 128 * (r["l0"] + r["n"])]
            else:
                lhsT = kT_sb[:, 128 * mi : 128 * mi + 128]
                rhs = qT_sb[:, 128 * r["l0"] : 128 * (r["l0"] + r["n"])]
            nc.tensor.matmul(
                ps[:, r["ps_col"] : r["ps_col"] + 128 * r["n"]],
                lhsT=lhsT,
                rhs=rhs,
                start=True,
                stop=True,
            )

    def emit_po(g):
        ps = g["ps"]
        dst = pT[:, g["pt_base"] : g["pt_base"] + g["cols"]]
        if g["eng"] == "act":
            nc.scalar.activation(
                dst, ps[:, : g["cols"]],
                mybir.ActivationFunctionType.Exp, scale=SCALE_ACT,
            )
        else:
            nc.vector.tensor_scalar(
                dst.bitcast(I16), ps[:, : g["cols"]],
                C1, C2, mybir.AluOpType.mult, mybir.AluOpType.add,
            )
        for r in g["runs"]:
            if r["diag"]:
                seg = pT[:, r["pt_col"] : r["pt_col"] + 128]
                nc.gpsimd.affine_select(
                    out=seg,
                    in_=seg,
                    pattern=[[1, 128]],
                    compare_op=mybir.AluOpType.is_ge,
                    fill=0.0,
                    base=0,
                    channel_multiplier=-1,
                )
        lp = g["lp"]
        for r in g["runs"]:
            mi = r["mi"]
            if mi == 0:
                state["po"] = psO.tile([128, 512], F32, name="po", tag="po")
            po = state["po"]
            odst = 128 * (r["l0"] - 4 * lp)
            nc.tensor.matmul(
                po[0:65, odst:512],
                lhsT=v_sb[:, 65 * mi : 65 * mi + 65],
                rhs=pT[:, r["pt_col"] : r["pt_col"] + 128 * r["n"]],
                start=(mi == 0),
                stop=r["o_last"],
                skip_group_check=True,
            )
            if r["o_last"]:
                if state["o_t"] is None:
                    state["o_t"] = o_pool.tile([128, L], BF16, name="ot", tag="ot")
                o_t = state["o_t"]
                # alternate the evacuation engine so neither queue backs up
                # (VectorE for the kernel's final window: its queue is
                # empty there, ScalarE still drains the forced-act exps)
                if lp % 2 == 0 or (s == NS - 1 and lp == 3):
                    nc.vector.tensor_copy(
                        o_t[0:65, 512 * lp : 512 * lp + 512], po[0:65, :]
                    )
                else:
                    nc.scalar.copy(
                        o_t[0:65, 512 * lp : 512 * lp + 512], po[0:65, :]
                    )
                if s == NS - 1:
                    # last slice: ship each window as it completes so the
                    # final DMA tail is one small window, not the slice
                    nc.sync.dma_start(
                        outT[s][:, 512 * lp : 512 * lp + 512],
                        o_t[0:65, 512 * lp : 512 * lp + 512],
                    )
                elif lp == 3:
                    nc.sync.dma_start(outT[s], o_t[0:65, :])

    # software pipeline: keep PIPE_DEPTH groups of S matmuls queued ahead
    # of the current group's exp/O flush so PE never waits on ScalarE
    pending = []
    nemit = 0
    for groups in WINDOWS:
        for g in groups:
            emit_s(g)
            nemit += 1
            if s == 0 and scratch is not None and nemit == PIPE_DEPTH:
                # fill slice 0's first-exp latency with dummy matmuls so
                # the HAM activity window never sees the PE idle
                po_d = psO.tile([128, 512], F32, name="po", tag="po")
                for _ in range(4):
                    nc.tensor.matmul(
                        po_d[:, :],
                        lhsT=scratch[:, 0:128],
                        rhs=scratch[:, :],
                        start=True,
                        stop=True,
                    )
            pending.append(g)
            if len(pending) > PIPE_DEPTH:
                emit_po(pending.pop(0))
    for g in pending:
        emit_po(g)


def _build():
    nc = bacc.Bacc(
        "TRN2",
        target_bir_lowering=False,
        debug=False,
        enable_asserts=True,
        num_devices=N_CORES,
    )
    qT = nc.dram_tensor("qT", [NS, E, L], BF16, kind="ExternalInput").ap()
    kT = nc.dram_tensor("kT", [NS, E, L], BF16, kind="ExternalInput").ap()
    v = nc.dram_tensor("v", [NS, L, E], BF16, kind="ExternalInput").ap()
    outT = nc.dram_tensor("outT", [NS, E + 1, L], BF16, kind="ExternalOutput").ap()

    with tile.TileContext(nc) as tc:
        with ExitStack() as ctx:

            def pool(name, bufs, space="SBUF"):
                return ctx.enter_context(
                    tc.tile_pool(name=name, bufs=bufs, space=space)
                )

            pools = (
                pool("io_q", 2),
                pool("io_k", 2),
                pool("io_v", 2),
                pool("pt", 2),
                pool("o", 2),
                pool("psS", 3, "PSUM"),
                pool("psO", 2, "PSUM"),
            )
            scratch = _emit_warmup(tc, pools)
            loads = _emit_loads(tc, pools, qT, kT, v, 0)
            for s in range(NS):
                nxt = (
                    _emit_loads(tc, pools, qT, kT, v, s + 1)
                    if s + 1 < NS else None
                )
                _emit_slice(tc, pools, loads, outT, s,
                            scratch if s == 0 else None)
                loads = nxt

    nc.compile()
    return nc


_NC_CACHE = {}


def _get_nc():
    if "nc" not in _NC_CACHE:
        _NC_CACHE["nc"] = _build()
    return _NC_CACHE["nc"]


def kernel(queries, keys, values, trace=False, tmpdir=None):
    nc = _get_nc()

    # shard: slice g = b*H + h; per-core slices [4c, 4c+4)
    qTf = np.ascontiguousarray(
        queries.transpose(0, 2, 3, 1).reshape(B * H, E, L)
    ).astype(BF16NP)
    kTf = np.ascontiguousarray(
        keys.transpose(0, 2, 3, 1).reshape(B * H, E, L)
    ).astype(BF16NP)
    vf = np.ascontiguousarray(
        values.transpose(0, 2, 1, 3).reshape(B * H, L, E)
    ).astype(BF16NP)

    in_maps = [
        {
            "qT": qTf[NS * c : NS * (c + 1)],
            "kT": kTf[NS * c : NS * (c + 1)],
            "v": vf[NS * c : NS * (c + 1)],
        }
        for c in range(N_CORES)
    ]

    res = run_bass_kernel_spmd(
        nc, in_maps, core_ids=list(range(N_CORES)), trace=trace, tmpdir=tmpdir
    )

    outT = np.concatenate([res.results[c]["outT"] for c in range(N_CORES)], axis=0)
    outT = outT.astype(np.float32)
    # outT: [B*H, 65, L]: rows 0..63 = O^T (unnormalized), row 64 = denom
    out = outT[:, :E, :] / outT[:, E : E + 1, :]
    # [B*H, E, L] -> [B, L, H, E]
    out = out.reshape(B, H, E, L).transpose(0, 3, 1, 2)
    out = np.ascontiguousarray(out, dtype=np.float32)
    if trace:
        kernel.last_exec_time_ns = res.exec_time_ns
    return out
